# revision 23
# baseline (speedup 1.0000x reference)
"""Trainium2 Bass kernel for the AttZAM attention-weight module.

Computation (full shapes):
    trans_q[b,j,a] = sum_k w_f[j,a,k] * emb_q[b,k]        b=256, j=256, a=128, k=256
    h[b,j,a]      = tanh(trans_q + b_f[j,a])
    g[b,j]        = sum_a h[b,j,a] * w_h[a,0]
    out[b,l]      = sum_j emb_iseq[b,l,j] * g[b,j]        l=1024

Sharding: the j axis (256) is split 8 ways (32 j's per core).  Each core
computes g[b, j_slice] for ALL b, then the partial contraction
sum_{j in slice} emb_iseq[b,l,j] * g[b,j] for all (b,l).  The host sums the
8 partial outputs.  No collectives needed.

Per-core kernel:
  Phase A (per j'): matmul lhsT=W_cT[k,ja] bf16, rhs=emb_q.T[k,b] -> psum
  [a=128, b=256]; tanh(+per-partition bias) on ScalarE -> h bf16; N=1 matmuls
  lhsT=h[:,b_chunk], rhs=w_h -> column j' of psum g[b=128, j=32].
  Phase B (per j'): D = diag(g[:, j']) via tensor_scalar_mul(identity, g-col);
  psum[b=128, l=512] += D.T @ E_perm[j', b_chunk, l_chunk], accumulating over
  all 32 j' in 4 held psum banks -> copy -> DMA out (bf16 partials).

Schedule notes (measured):
  - HWDGE dma_start costs ~0.65us issue time on its engine and each DMA's
    completion semaphore adds ~1.5us receipt latency; per-ring FIFO order
    chains those.  So: small inputs are packed into ONE head DMA (fp32 bias
    bit-packed into the bf16 tensor, bitcast on-chip), W rides the Sync ring
    in two more DMAs, and the 12 E-group DMAs ride the GpSimd SWDGE ring so
    neither stream blocks the other.
  - Phase A groups lead phase B groups by 3 in the PE stream (engines are
    in-order), so the kernel tail after the last E tile is just one tiny
    B-group + psum copies + output DMAs.
"""

import sys

import numpy as np
import ml_dtypes

sys.path.insert(0, "/opt/trn_rl_repo")

import concourse.bass as bass  # noqa: E402,F401
import concourse.mybir as mybir  # noqa: E402
import concourse.tile as tile  # noqa: E402
from concourse import bacc  # noqa: E402
from concourse.bass_utils import run_bass_kernel_spmd  # noqa: E402
from concourse.masks import make_identity  # noqa: E402

N_CORES = 8
BSZ, MAX_LEN, D, D_ATTN = 256, 1024, 256, 128
JS = D // N_CORES          # 32 j's per core
JA = JS * D_ATTN           # 4096 rows of the per-core W slice
P = 128                    # partitions
KC = D // P                # 2 k-chunks
NB = BSZ // P              # 2 b-chunks
JG = 4                     # max j's per group
LCH = 512                  # l-chunk (one fp32 psum bank)
NL = MAX_LEN // LCH        # 2 l-chunks

GROUP_SIZES = [1, 1, 2, 4, 4, 4, 4, 4, 4, 2, 1, 1]
assert sum(GROUP_SIZES) == JS
NGRP = len(GROUP_SIZES)
GROUP_STARTS = [sum(GROUP_SIZES[:i]) for i in range(NGRP)]
LEAD = 3                   # phase-A groups emitted ahead of phase-B groups

HEAD_WJ = GROUP_SIZES[0]   # j's whose W slice rides in the head DMA
# head tensor column layout (bf16 cols): q (KC*BSZ) | wh | pad | bias-as-bf16
Q_COLS = KC * BSZ
BIAS_OFF = Q_COLS + 2      # 4-byte aligned
W_OFF = BIAS_OFF + 2 * JS
HEADC = W_OFF + KC * HEAD_WJ * D_ATTN

BF16 = mybir.dt.bfloat16
F32 = mybir.dt.float32
bf16_np = ml_dtypes.bfloat16

_CACHED_NC = None


def build_nc():
    nc = bacc.Bacc(
        "TRN2",
        target_bir_lowering=False,
        debug=False,
        num_devices=N_CORES,
    )

    head = nc.dram_tensor("head", [P, HEADC], BF16, kind="ExternalInput")
    w_rest = nc.dram_tensor(
        "w_rest", [P, KC, (JS - HEAD_WJ) * D_ATTN], BF16, kind="ExternalInput"
    )
    e2 = nc.dram_tensor("e2", [P, NB, JS, MAX_LEN], BF16, kind="ExternalInput")
    out = nc.dram_tensor("out", [BSZ, MAX_LEN], BF16, kind="ExternalOutput")

    with tile.TileContext(nc) as tc:
        with (
            tc.tile_pool(name="const", bufs=1) as cpool,
            tc.tile_pool(name="wpool", bufs=3) as wpool,
            tc.tile_pool(name="epool", bufs=5) as epool,
            tc.tile_pool(name="hpool", bufs=4) as hpool,
            tc.tile_pool(name="dpool", bufs=4) as dpool,
            tc.tile_pool(name="opool", bufs=2) as opool,
            tc.tile_pool(name="psA", bufs=2, space="PSUM") as psa_pool,
            tc.tile_pool(name="psG", bufs=1, space="PSUM") as psg_pool,
            tc.tile_pool(name="psB", bufs=1, space="PSUM") as psb_pool,
        ):
            head_sb = cpool.tile([P, HEADC], BF16, tag="head", name="head_sb")
            nc.sync.dma_start(out=head_sb, in_=head[:, :])

            q_sb = [head_sb[:, kc * BSZ : (kc + 1) * BSZ] for kc in range(KC)]
            wh_sb = head_sb[:, Q_COLS : Q_COLS + 1]
            bias_sb = head_sb[:, BIAS_OFF : BIAS_OFF + 2 * JS].bitcast(F32)

            ident = cpool.tile([P, P], BF16, tag="ident", name="ident")
            make_identity(nc, ident)

            # PE warm-up while the first data DMAs are in flight, plus a
            # zeroed tile reused for keep-warm filler matmuls later (any PE
            # idle window >~3.4us re-throttles the clock gate to 1.2 GHz).
            wz = cpool.tile([P, P], BF16, tag="wz", name="wz")
            nc.vector.memset(wz, 0.0)
            wps = psa_pool.tile([64, P], F32, tag="psA", name="warm_ps")
            for _ in range(8):
                nc.tensor.matmul(wps, wz[:, :64], wz, start=True, stop=True)

            g_sb = [
                cpool.tile([P, JS], F32, tag=f"g{bc}", name=f"g_sb{bc}")
                for bc in range(NB)
            ]
            g_ps = [
                psg_pool.tile([P, JS], F32, tag=f"gps{bc}", name=f"g_ps{bc}")
                for bc in range(NB)
            ]
            ps_out = [
                [
                    psb_pool.tile([P, LCH], F32, tag=f"psB{bc}_{lc}", name=f"psB{bc}_{lc}")
                    for lc in range(NL)
                ]
                for bc in range(NB)
            ]
            etiles = [None] * NGRP
            wtiles = [None] * NGRP

            def w_lhsT(i, kc, jj):
                if i == 0:
                    off = W_OFF + kc * HEAD_WJ * D_ATTN + jj * D_ATTN
                    return head_sb[:, off : off + D_ATTN]
                return wtiles[i][:, kc, jj * D_ATTN : (jj + 1) * D_ATTN]

            def emit_a(i):
                jp0, gsz = GROUP_STARTS[i], GROUP_SIZES[i]
                # E prefetch for this group rides the GpSimd SWDGE ring.
                # Every group gets its own resident buffer (no slot reuse), so
                # all E DMAs are issued up-front and delivery never feeds back
                # on PE progress.
                et = epool.tile(
                    [P, NB, gsz, MAX_LEN],
                    BF16,
                    tag=f"e{gsz}",
                    bufs=sum(1 for g in GROUP_SIZES if g == gsz),
                    name="et",
                )
                nc.gpsimd.dma_start(out=et, in_=e2[:, :, jp0 : jp0 + gsz, :])
                etiles[i] = et
                if i > 0:
                    wt = wpool.tile([P, KC, JG * D_ATTN], BF16, tag="w", name="w_g")
                    off0 = (jp0 - HEAD_WJ) * D_ATTN
                    nc.sync.dma_start(
                        out=wt[:, :, : gsz * D_ATTN],
                        in_=w_rest[:, :, off0 : off0 + gsz * D_ATTN],
                    )
                    wtiles[i] = wt
                for jj in range(gsz):
                    jp = jp0 + jj
                    ps = psa_pool.tile([P, BSZ], F32, tag="psA", name="psA")
                    for kc in range(KC):
                        nc.tensor.matmul(
                            ps,
                            w_lhsT(i, kc, jj),
                            q_sb[kc],
                            start=(kc == 0),
                            stop=(kc == KC - 1),
                        )
                    h = hpool.tile([P, BSZ], BF16, tag="h", name="h")
                    nc.scalar.activation(
                        h,
                        ps,
                        mybir.ActivationFunctionType.Tanh,
                        bias=bias_sb[:, jp : jp + 1],
                    )
                    for bc in range(NB):
                        nc.tensor.matmul(
                            g_ps[bc][:, jp : jp + 1],
                            h[:, bc * P : (bc + 1) * P],
                            wh_sb,
                            start=True,
                            stop=True,
                        )
                for bc in range(NB):
                    nc.vector.tensor_copy(
                        g_sb[bc][:, jp0 : jp0 + gsz],
                        g_ps[bc][:, jp0 : jp0 + gsz],
                    )

            def emit_b(i):
                jp0, gsz = GROUP_STARTS[i], GROUP_SIZES[i]
                et = etiles[i]
                for jj in range(gsz):
                    jp = jp0 + jj
                    ds = []
                    for bc in range(NB):
                        d = dpool.tile([P, P], BF16, tag=f"d{bc}", name=f"d{bc}")
                        nc.vector.tensor_scalar_mul(d, ident, g_sb[bc][:, jp : jp + 1])
                        ds.append(d)
                    for bc in range(NB):
                        for lc in range(NL):
                            nc.tensor.matmul(
                                ps_out[bc][lc],
                                ds[bc],
                                et[:, bc, jj, lc * LCH : (lc + 1) * LCH],
                                start=(jp == 0),
                                stop=(jp == JS - 1),
                            )

            for i in range(NGRP + LEAD):
                if i < NGRP:
                    emit_a(i)
                if i >= LEAD:
                    emit_b(i - LEAD)

            # ---- tail: psum -> sbuf (bf16) -> dram, one DMA per b-chunk ----
            for bc in range(NB):
                o = opool.tile([P, MAX_LEN], BF16, tag=f"o{bc}", name=f"o{bc}")
                nc.vector.tensor_copy(o[:, :LCH], ps_out[bc][0])
                nc.scalar.copy(o[:, LCH:], ps_out[bc][1])
                nc.sync.dma_start(out=out[bc * P : (bc + 1) * P, :], in_=o)

    nc.compile()
    return nc


def get_nc():
    global _CACHED_NC
    if _CACHED_NC is None:
        _CACHED_NC = build_nc()
    return _CACHED_NC


def make_in_maps(emb_q, emb_iseq, w_f, b_f, w_h):
    """Host-side shard + layout + bf16 cast.  Returns list of per-core dicts."""
    q_t = np.ascontiguousarray(emb_q.astype(np.float32).T).astype(bf16_np)  # [k, b]
    qpart = q_t.reshape(KC, P, BSZ).transpose(1, 0, 2).reshape(P, KC * BSZ)
    wh_col = w_h.astype(bf16_np).reshape(1, D_ATTN).T  # [128, 1]
    in_maps = []
    for c in range(N_CORES):
        js, je = c * JS, (c + 1) * JS
        w_slice = w_f[js:je].reshape(JA, D)                       # [ja, k]
        w_t = w_slice.T.astype(bf16_np)                           # [k, ja]
        w2 = np.ascontiguousarray(
            w_t.reshape(KC, P, JA).transpose(1, 0, 2)
        )                                                         # [128, KC, ja]
        bias32 = np.ascontiguousarray(
            b_f[js:je].T.astype(np.float32)
        )                                                         # [a, j'] fp32
        bias_as_bf16 = bias32.view(bf16_np)                       # [128, 64]

        head = np.zeros((P, HEADC), dtype=bf16_np)
        head[:, :Q_COLS] = qpart
        head[:, Q_COLS : Q_COLS + 1] = wh_col
        head[:, BIAS_OFF : BIAS_OFF + 2 * JS] = bias_as_bf16
        head[:, W_OFF:] = w2[:, :, : HEAD_WJ * D_ATTN].reshape(P, KC * HEAD_WJ * D_ATTN)

        w_rest = np.ascontiguousarray(w2[:, :, HEAD_WJ * D_ATTN :])

        e_perm = emb_iseq[:, :, js:je].transpose(0, 2, 1)         # [b, j', l]
        e2 = np.ascontiguousarray(
            e_perm.astype(bf16_np)
            .reshape(NB, P, JS, MAX_LEN)
            .transpose(1, 0, 2, 3)
        )                                                         # [128, NB, j', l]
        in_maps.append({"head": head, "w_rest": w_rest, "e2": e2})
    return in_maps


def run(in_maps, trace=False, **kwargs):
    nc = get_nc()
    return run_bass_kernel_spmd(
        nc, in_maps, core_ids=list(range(N_CORES)), trace=trace, **kwargs
    )


def kernel(emb_q, emb_iseq, w_f, b_f, w_h):
    emb_q, emb_iseq, w_f, b_f, w_h = (
        np.asarray(x) for x in (emb_q, emb_iseq, w_f, b_f, w_h)
    )
    in_maps = make_in_maps(emb_q, emb_iseq, w_f, b_f, w_h)
    res = run(in_maps, trace=False)
    partial = np.zeros((BSZ, MAX_LEN), dtype=np.float32)
    for r in res.results:
        partial += r["out"].astype(np.float32)
    return partial


# revision 24
# speedup vs baseline: 1.2446x; 1.2446x over previous
"""Trainium2 Bass kernel for the AttZAM attention-weight module.

Computation (full shapes):
    trans_q[b,j,a] = sum_k w_f[j,a,k] * emb_q[b,k]        b=256, j=256, a=128, k=256
    h[b,j,a]      = tanh(trans_q + b_f[j,a])
    g[b,j]        = sum_a h[b,j,a] * w_h[a,0]
    out[b,l]      = sum_j emb_iseq[b,l,j] * g[b,j]        l=1024

Sharding: the j axis (256) is split 8 ways (32 j's per core).  Each core
computes g[b, j_slice] for ALL b, then the partial contraction
sum_{j in slice} emb_iseq[b,l,j] * g[b,j] for all (b,l).  The host sums the
8 partial outputs.  No collectives needed.

Per-core kernel:
  Phase A (per j'): matmul lhsT=W_cT[k,ja] bf16, rhs=emb_q.T[k,b] -> psum
  [a=128, b=256]; tanh(+per-partition bias) on ScalarE -> h bf16; N=1 matmuls
  lhsT=h[:,b_chunk], rhs=w_h -> column j' of psum g[b=128, j=32].
  Phase B (per j'): D = diag(g[:, j']) via tensor_scalar_mul(identity, g-col);
  psum[b=128, l=512] += D.T @ E_perm[j', b_chunk, l_chunk], accumulating over
  all 32 j' in 4 held psum banks -> copy -> DMA out (bf16 partials).

Schedule notes (measured on HW):
  - Each dma_start costs ~0.65us of issue time on its engine and ~1.3us of
    completion-semaphore latency, chained per-ring FIFO.  So small inputs are
    packed into ONE head DMA (fp32 bias bit-packed into the bf16 tensor,
    bitcast on-chip), W rides the Sync ring in two more DMAs, and the E-group
    DMAs ride the GpSimd SWDGE ring so neither stream blocks the other.
  - The last two (1-j') E groups are prefetched at kernel start on the Scalar
    HWDGE ring into resident tiles, so the kernel tail after the final
    streamed E tile is one tiny B-group with no DMA wait.
  - Phase A groups lead phase B groups by 3 in the PE stream (engines are
    in-order), with small leading groups so B starts early.
"""

import sys

import numpy as np
import ml_dtypes

sys.path.insert(0, "/opt/trn_rl_repo")

import concourse.bass as bass  # noqa: E402,F401
import concourse.mybir as mybir  # noqa: E402
import concourse.tile as tile  # noqa: E402
from concourse import bacc  # noqa: E402
from concourse.bass_utils import run_bass_kernel_spmd  # noqa: E402
from concourse.masks import make_identity  # noqa: E402

N_CORES = 8
BSZ, MAX_LEN, D, D_ATTN = 256, 1024, 256, 128
JS = D // N_CORES          # 32 j's per core
JA = JS * D_ATTN           # 4096 rows of the per-core W slice
P = 128                    # partitions
KC = D // P                # 2 k-chunks
NB = BSZ // P              # 2 b-chunks
JG = 4                     # max j's per group
LCH = 512                  # l-chunk (one fp32 psum bank)
NL = MAX_LEN // LCH        # 2 l-chunks

GROUP_SIZES = [1, 1, 2, 4, 4, 4, 4, 4, 4, 2, 1, 1]
assert sum(GROUP_SIZES) == JS
NGRP = len(GROUP_SIZES)
GROUP_STARTS = [sum(GROUP_SIZES[:i]) for i in range(NGRP)]
LEAD = 3                   # phase-A groups emitted ahead of phase-B groups
N_PRE = 2                  # trailing groups prefetched at kernel start

HEAD_WJ = 4                # j's whose W slice rides in the head DMA
MID_WJ = 12                # j's in the w_mid DMA (rest go in w_tail)
# head tensor column layout (bf16 cols): q (KC*BSZ) | wh | pad | bias-as-bf16
Q_COLS = KC * BSZ
BIAS_OFF = Q_COLS + 2      # 4-byte aligned
W_OFF = BIAS_OFF + 2 * JS
HEADC = W_OFF + KC * HEAD_WJ * D_ATTN

BF16 = mybir.dt.bfloat16
F32 = mybir.dt.float32
bf16_np = ml_dtypes.bfloat16

_CACHED_NC = None


def build_nc():
    nc = bacc.Bacc(
        "TRN2",
        target_bir_lowering=False,
        debug=False,
        num_devices=N_CORES,
    )

    head = nc.dram_tensor("head", [P, HEADC], BF16, kind="ExternalInput")
    w_mid = nc.dram_tensor("w_mid", [P, KC, MID_WJ * D_ATTN], BF16, kind="ExternalInput")
    w_tail = nc.dram_tensor(
        "w_tail", [P, KC, (JS - HEAD_WJ - MID_WJ) * D_ATTN], BF16, kind="ExternalInput"
    )
    e2 = nc.dram_tensor("e2", [P, NB, JS, MAX_LEN], BF16, kind="ExternalInput")
    out = nc.dram_tensor("out", [BSZ, MAX_LEN], BF16, kind="ExternalOutput")

    with tile.TileContext(nc) as tc:
        with (
            tc.tile_pool(name="const", bufs=1) as cpool,
            tc.tile_pool(name="epool", bufs=5) as epool,
            tc.tile_pool(name="hpool", bufs=4) as hpool,
            tc.tile_pool(name="dpool", bufs=4) as dpool,
            tc.tile_pool(name="opool", bufs=2) as opool,
            tc.tile_pool(name="psA", bufs=2, space="PSUM") as psa_pool,
            tc.tile_pool(name="psG", bufs=1, space="PSUM") as psg_pool,
            tc.tile_pool(name="psB", bufs=1, space="PSUM") as psb_pool,
        ):
            head_sb = cpool.tile([P, HEADC], BF16, tag="head", name="head_sb")
            nc.sync.dma_start(out=head_sb, in_=head[:, :])

            # prefetch the trailing tiny E groups on the (otherwise idle at
            # kernel start) Scalar HWDGE ring; consumed at the very end.
            pre_tiles = {}
            for i in range(NGRP - N_PRE, NGRP):
                jp0, gsz = GROUP_STARTS[i], GROUP_SIZES[i]
                pt = cpool.tile(
                    [P, NB, gsz, MAX_LEN], BF16, tag=f"epre{i}", name=f"epre{i}"
                )
                nc.scalar.dma_start(out=pt, in_=e2[:, :, jp0 : jp0 + gsz, :])
                pre_tiles[i] = pt

            wm_sb = cpool.tile([P, KC, MID_WJ * D_ATTN], BF16, tag="wm", name="wm_sb")
            nc.sync.dma_start(out=wm_sb, in_=w_mid[:, :, :])
            wt_sb = cpool.tile(
                [P, KC, (JS - HEAD_WJ - MID_WJ) * D_ATTN], BF16, tag="wt", name="wt_sb"
            )
            nc.sync.dma_start(out=wt_sb, in_=w_tail[:, :, :])

            q_sb = [head_sb[:, kc * BSZ : (kc + 1) * BSZ] for kc in range(KC)]
            wh_sb = head_sb[:, Q_COLS : Q_COLS + 1]
            bias_sb = head_sb[:, BIAS_OFF : BIAS_OFF + 2 * JS].bitcast(F32)

            def w_lhsT(kc, jp):
                if jp < HEAD_WJ:
                    off = W_OFF + kc * HEAD_WJ * D_ATTN + jp * D_ATTN
                    return head_sb[:, off : off + D_ATTN]
                if jp < HEAD_WJ + MID_WJ:
                    off = (jp - HEAD_WJ) * D_ATTN
                    return wm_sb[:, kc, off : off + D_ATTN]
                off = (jp - HEAD_WJ - MID_WJ) * D_ATTN
                return wt_sb[:, kc, off : off + D_ATTN]

            ident = cpool.tile([P, P], BF16, tag="ident", name="ident")
            make_identity(nc, ident)

            g_sb = [
                cpool.tile([P, JS], F32, tag=f"g{bc}", name=f"g_sb{bc}")
                for bc in range(NB)
            ]
            g_ps = [
                psg_pool.tile([P, JS], F32, tag=f"gps{bc}", name=f"g_ps{bc}")
                for bc in range(NB)
            ]
            ps_out = [
                [
                    psb_pool.tile([P, LCH], F32, tag=f"psB{bc}_{lc}", name=f"psB{bc}_{lc}")
                    for lc in range(NL)
                ]
                for bc in range(NB)
            ]
            etiles = [None] * NGRP

            def emit_a(i):
                jp0, gsz = GROUP_STARTS[i], GROUP_SIZES[i]
                if i < NGRP - N_PRE:
                    # E for this group rides the GpSimd SWDGE ring
                    et = epool.tile([P, NB, JG, MAX_LEN], BF16, tag="e", name="et")
                    nc.gpsimd.dma_start(
                        out=et[:, :, :gsz, :], in_=e2[:, :, jp0 : jp0 + gsz, :]
                    )
                    etiles[i] = et
                else:
                    etiles[i] = pre_tiles[i]
                for jj in range(gsz):
                    jp = jp0 + jj
                    ps = psa_pool.tile([P, BSZ], F32, tag="psA", name="psA")
                    for kc in range(KC):
                        nc.tensor.matmul(
                            ps,
                            w_lhsT(kc, jp),
                            q_sb[kc],
                            start=(kc == 0),
                            stop=(kc == KC - 1),
                        )
                    h = hpool.tile([P, BSZ], BF16, tag="h", name="h")
                    nc.scalar.activation(
                        h,
                        ps,
                        mybir.ActivationFunctionType.Tanh,
                        bias=bias_sb[:, jp : jp + 1],
                    )
                    for bc in range(NB):
                        nc.tensor.matmul(
                            g_ps[bc][:, jp : jp + 1],
                            h[:, bc * P : (bc + 1) * P],
                            wh_sb,
                            start=True,
                            stop=True,
                        )
                for bc in range(NB):
                    nc.vector.tensor_copy(
                        g_sb[bc][:, jp0 : jp0 + gsz],
                        g_ps[bc][:, jp0 : jp0 + gsz],
                    )

            def emit_b(i):
                jp0, gsz = GROUP_STARTS[i], GROUP_SIZES[i]
                et = etiles[i]
                for jj in range(gsz):
                    jp = jp0 + jj
                    ds = []
                    for bc in range(NB):
                        d = dpool.tile([P, P], BF16, tag=f"d{bc}", name=f"d{bc}")
                        nc.vector.tensor_scalar_mul(d, ident, g_sb[bc][:, jp : jp + 1])
                        ds.append(d)
                    for bc in range(NB):
                        for lc in range(NL):
                            nc.tensor.matmul(
                                ps_out[bc][lc],
                                ds[bc],
                                et[:, bc, jj, lc * LCH : (lc + 1) * LCH],
                                start=(jp == 0),
                                stop=(jp == JS - 1),
                            )

            for i in range(NGRP + LEAD):
                if i < NGRP:
                    emit_a(i)
                if i >= LEAD:
                    emit_b(i - LEAD)

            # ---- tail: psum -> sbuf (bf16) -> dram, split per (bc, lc) ----
            for bc in range(NB):
                for lc in range(NL):
                    o = opool.tile([P, LCH], BF16, tag=f"o{bc}_{lc}", name=f"o{bc}_{lc}")
                    if bc == 0:
                        nc.vector.tensor_copy(o, ps_out[bc][lc])
                    else:
                        nc.scalar.copy(o, ps_out[bc][lc])
                    nc.sync.dma_start(
                        out=out[bc * P : (bc + 1) * P, lc * LCH : (lc + 1) * LCH],
                        in_=o,
                    )

    nc.compile()
    return nc


def get_nc():
    global _CACHED_NC
    if _CACHED_NC is None:
        _CACHED_NC = build_nc()
    return _CACHED_NC


def make_in_maps(emb_q, emb_iseq, w_f, b_f, w_h):
    """Host-side shard + layout + bf16 cast.  Returns list of per-core dicts."""
    q_t = np.ascontiguousarray(emb_q.astype(np.float32).T).astype(bf16_np)  # [k, b]
    qpart = q_t.reshape(KC, P, BSZ).transpose(1, 0, 2).reshape(P, KC * BSZ)
    wh_col = w_h.astype(bf16_np).reshape(1, D_ATTN).T  # [128, 1]
    in_maps = []
    for c in range(N_CORES):
        js, je = c * JS, (c + 1) * JS
        w_slice = w_f[js:je].reshape(JA, D)                       # [ja, k]
        w_t = w_slice.T.astype(bf16_np)                           # [k, ja]
        w2 = np.ascontiguousarray(
            w_t.reshape(KC, P, JA).transpose(1, 0, 2)
        )                                                         # [128, KC, ja]
        bias32 = np.ascontiguousarray(
            b_f[js:je].T.astype(np.float32)
        )                                                         # [a, j'] fp32
        bias_as_bf16 = bias32.view(bf16_np)                       # [128, 64]

        head = np.zeros((P, HEADC), dtype=bf16_np)
        head[:, :Q_COLS] = qpart
        head[:, Q_COLS : Q_COLS + 1] = wh_col
        head[:, BIAS_OFF : BIAS_OFF + 2 * JS] = bias_as_bf16
        head[:, W_OFF:] = w2[:, :, : HEAD_WJ * D_ATTN].reshape(P, KC * HEAD_WJ * D_ATTN)

        w_mid = np.ascontiguousarray(
            w2[:, :, HEAD_WJ * D_ATTN : (HEAD_WJ + MID_WJ) * D_ATTN]
        )
        w_tail = np.ascontiguousarray(w2[:, :, (HEAD_WJ + MID_WJ) * D_ATTN :])

        e_perm = emb_iseq[:, :, js:je].transpose(0, 2, 1)         # [b, j', l]
        e2 = np.ascontiguousarray(
            e_perm.astype(bf16_np)
            .reshape(NB, P, JS, MAX_LEN)
            .transpose(1, 0, 2, 3)
        )                                                         # [128, NB, j', l]
        in_maps.append({"head": head, "w_mid": w_mid, "w_tail": w_tail, "e2": e2})
    return in_maps


def run(in_maps, trace=False, **kwargs):
    nc = get_nc()
    return run_bass_kernel_spmd(
        nc, in_maps, core_ids=list(range(N_CORES)), trace=trace, **kwargs
    )


def kernel(emb_q, emb_iseq, w_f, b_f, w_h):
    emb_q, emb_iseq, w_f, b_f, w_h = (
        np.asarray(x) for x in (emb_q, emb_iseq, w_f, b_f, w_h)
    )
    in_maps = make_in_maps(emb_q, emb_iseq, w_f, b_f, w_h)
    res = run(in_maps, trace=False)
    partial = np.zeros((BSZ, MAX_LEN), dtype=np.float32)
    for r in res.results:
        partial += r["out"].astype(np.float32)
    return partial


# revision 30
# speedup vs baseline: 1.2608x; 1.0130x over previous
"""Trainium2 Bass kernel for the AttZAM attention-weight module.

Computation (full shapes):
    trans_q[b,j,a] = sum_k w_f[j,a,k] * emb_q[b,k]        b=256, j=256, a=128, k=256
    h[b,j,a]      = tanh(trans_q + b_f[j,a])
    g[b,j]        = sum_a h[b,j,a] * w_h[a,0]
    out[b,l]      = sum_j emb_iseq[b,l,j] * g[b,j]        l=1024

Sharding: the j axis (256) is split 8 ways (32 j's per core).  Each core
computes g[b, j_slice] for ALL b, then the partial contraction
sum_{j in slice} emb_iseq[b,l,j] * g[b,j] for all (b,l).  The host sums the
8 partial outputs.  No collectives needed.

Per-core kernel:
  Phase A (per j'): matmul lhsT=W_cT[k,ja] bf16, rhs=emb_q.T[k,b] -> psum
  [a=128, b=256]; tanh(+per-partition bias) on ScalarE -> h bf16; N=1 matmuls
  lhsT=h[:,b_chunk], rhs=w_h -> column j' of psum g[b=128, j=32].
  Phase B (per j'): D = diag(g[:, j']) via tensor_scalar_mul(identity, g-col);
  psum[b=128, l=512] += D.T @ E_perm[j', b_chunk, l_chunk], accumulating over
  all 32 j' in 4 held psum banks -> copy -> DMA out (bf16 partials).

Schedule notes (measured on HW):
  - Each dma_start costs ~0.65us of issue time on its engine and ~1.3us of
    completion-semaphore latency, chained per-ring FIFO.  So small inputs are
    packed into ONE head DMA (fp32 bias bit-packed into the bf16 tensor,
    bitcast on-chip), W rides the Sync ring in two more DMAs, and the E-group
    DMAs ride the GpSimd SWDGE ring so neither stream blocks the other.
  - The last two (1-j') E groups are prefetched at kernel start on the Scalar
    HWDGE ring into resident tiles, so the kernel tail after the final
    streamed E tile is one tiny B-group with no DMA wait.
  - Phase A groups lead phase B groups by 3 in the PE stream (engines are
    in-order), with small leading groups so B starts early.
"""

import sys

import numpy as np
import ml_dtypes

sys.path.insert(0, "/opt/trn_rl_repo")

import concourse.bass as bass  # noqa: E402,F401
import concourse.mybir as mybir  # noqa: E402
import concourse.tile as tile  # noqa: E402
from concourse import bacc  # noqa: E402
from concourse.bass_utils import run_bass_kernel_spmd  # noqa: E402
from concourse.masks import make_identity  # noqa: E402

N_CORES = 8
BSZ, MAX_LEN, D, D_ATTN = 256, 1024, 256, 128
JS = D // N_CORES          # 32 j's per core
JA = JS * D_ATTN           # 4096 rows of the per-core W slice
P = 128                    # partitions
KC = D // P                # 2 k-chunks
NB = BSZ // P              # 2 b-chunks
JG = 4                     # max j's per group
LCH = 512                  # l-chunk (one fp32 psum bank)
NL = MAX_LEN // LCH        # 2 l-chunks

GROUP_SIZES = [1, 1, 2, 4, 4, 4, 5, 4, 4, 1, 1, 1]
assert sum(GROUP_SIZES) == JS
NGRP = len(GROUP_SIZES)
GROUP_STARTS = [sum(GROUP_SIZES[:i]) for i in range(NGRP)]
MAX_G = max(GROUP_SIZES)
LEAD = 3                   # phase-A groups emitted ahead of phase-B groups
N_PRE = 2                  # trailing groups prefetched at kernel start

HEAD_WJ = 4                # j's whose W slice rides in the head DMA
MID_WJ = 12                # j's in the w_mid DMA (rest go in w_tail)
# head layout (bf16 cols): q (KC*BSZ) | wh | pad | bias-as-bf16 | w | identity
Q_COLS = KC * BSZ
BIAS_OFF = Q_COLS + 2      # 4-byte aligned
W_OFF = BIAS_OFF + 2 * JS
ID_OFF = W_OFF + KC * HEAD_WJ * D_ATTN
HEADC = ID_OFF + P

BF16 = mybir.dt.bfloat16
F32 = mybir.dt.float32
bf16_np = ml_dtypes.bfloat16

_CACHED_NC = None


def build_nc():
    nc = bacc.Bacc(
        "TRN2",
        target_bir_lowering=False,
        debug=False,
        num_devices=N_CORES,
    )

    head = nc.dram_tensor("head", [P, HEADC], BF16, kind="ExternalInput")
    w_mid = nc.dram_tensor("w_mid", [P, KC, MID_WJ * D_ATTN], BF16, kind="ExternalInput")
    w_tail = nc.dram_tensor(
        "w_tail", [P, KC, (JS - HEAD_WJ - MID_WJ) * D_ATTN], BF16, kind="ExternalInput"
    )
    e2 = nc.dram_tensor("e2", [P, NB, JS, MAX_LEN], BF16, kind="ExternalInput")
    out = nc.dram_tensor("out", [BSZ, MAX_LEN], BF16, kind="ExternalOutput")

    with tile.TileContext(nc) as tc:
        with (
            tc.tile_pool(name="const", bufs=1) as cpool,
            tc.tile_pool(name="epool", bufs=5) as epool,
            tc.tile_pool(name="hpool", bufs=4) as hpool,
            tc.tile_pool(name="dpool", bufs=4) as dpool,
            tc.tile_pool(name="opool", bufs=2) as opool,
            tc.tile_pool(name="psA", bufs=2, space="PSUM") as psa_pool,
            tc.tile_pool(name="psG", bufs=1, space="PSUM") as psg_pool,
            tc.tile_pool(name="psB", bufs=1, space="PSUM") as psb_pool,
        ):
            head_sb = cpool.tile([P, HEADC], BF16, tag="head", name="head_sb")
            nc.sync.dma_start(out=head_sb, in_=head[:, :])

            wm_sb = cpool.tile([P, KC, MID_WJ * D_ATTN], BF16, tag="wm", name="wm_sb")
            nc.sync.dma_start(out=wm_sb, in_=w_mid[:, :, :])
            wt_sb = cpool.tile(
                [P, KC, (JS - HEAD_WJ - MID_WJ) * D_ATTN], BF16, tag="wt", name="wt_sb"
            )
            nc.sync.dma_start(out=wt_sb, in_=w_tail[:, :, :])

            q_sb = [head_sb[:, kc * BSZ : (kc + 1) * BSZ] for kc in range(KC)]
            wh_sb = head_sb[:, Q_COLS : Q_COLS + 1]
            bias_sb = head_sb[:, BIAS_OFF : BIAS_OFF + 2 * JS].bitcast(F32)

            def w_lhsT(kc, jp):
                if jp < HEAD_WJ:
                    off = W_OFF + kc * HEAD_WJ * D_ATTN + jp * D_ATTN
                    return head_sb[:, off : off + D_ATTN]
                if jp < HEAD_WJ + MID_WJ:
                    off = (jp - HEAD_WJ) * D_ATTN
                    return wm_sb[:, kc, off : off + D_ATTN]
                off = (jp - HEAD_WJ - MID_WJ) * D_ATTN
                return wt_sb[:, kc, off : off + D_ATTN]

            ident = head_sb[:, ID_OFF : ID_OFF + P]

            # trailing tiny E groups, prefetched on the Scalar HWDGE ring
            # (emitted inside the loop after the first tanh so their data
            # doesn't compete with the critical head/w_mid transfers);
            # consumed at the very end with no DMA wait.
            pre_tiles = {}

            g_sb = [
                cpool.tile([P, JS], F32, tag=f"g{bc}", name=f"g_sb{bc}")
                for bc in range(NB)
            ]
            g_ps = [
                psg_pool.tile([P, JS], F32, tag=f"gps{bc}", name=f"g_ps{bc}")
                for bc in range(NB)
            ]
            ps_out = [
                [
                    psb_pool.tile([P, LCH], F32, tag=f"psB{bc}_{lc}", name=f"psB{bc}_{lc}")
                    for lc in range(NL)
                ]
                for bc in range(NB)
            ]
            etiles = [None] * NGRP

            def emit_a(i):
                jp0, gsz = GROUP_STARTS[i], GROUP_SIZES[i]
                if i < NGRP - N_PRE:
                    # E for this group rides the GpSimd SWDGE ring
                    et = epool.tile([P, NB, MAX_G, MAX_LEN], BF16, tag="e", name="et")
                    nc.gpsimd.dma_start(
                        out=et[:, :, :gsz, :], in_=e2[:, :, jp0 : jp0 + gsz, :]
                    )
                    etiles[i] = et
                else:
                    etiles[i] = pre_tiles[i]
                for jj in range(gsz):
                    jp = jp0 + jj
                    ps = psa_pool.tile([P, BSZ], F32, tag="psA", name="psA")
                    for kc in range(KC):
                        nc.tensor.matmul(
                            ps,
                            w_lhsT(kc, jp),
                            q_sb[kc],
                            start=(kc == 0),
                            stop=(kc == KC - 1),
                        )
                    h = hpool.tile([P, BSZ], BF16, tag="h", name="h")
                    nc.scalar.activation(
                        h,
                        ps,
                        mybir.ActivationFunctionType.Tanh,
                        bias=bias_sb[:, jp : jp + 1],
                    )
                    for bc in range(NB):
                        nc.tensor.matmul(
                            g_ps[bc][:, jp : jp + 1],
                            h[:, bc * P : (bc + 1) * P],
                            wh_sb,
                            start=True,
                            stop=True,
                        )
                for bc in range(NB):
                    nc.vector.tensor_copy(
                        g_sb[bc][:, jp0 : jp0 + gsz],
                        g_ps[bc][:, jp0 : jp0 + gsz],
                    )

            def emit_b(i):
                jp0, gsz = GROUP_STARTS[i], GROUP_SIZES[i]
                et = etiles[i]
                for jj in range(gsz):
                    jp = jp0 + jj
                    ds = []
                    for bc in range(NB):
                        d = dpool.tile([P, P], BF16, tag=f"d{bc}", name=f"d{bc}")
                        nc.vector.tensor_scalar_mul(d, ident, g_sb[bc][:, jp : jp + 1])
                        ds.append(d)
                    for bc in range(NB):
                        for lc in range(NL):
                            nc.tensor.matmul(
                                ps_out[bc][lc],
                                ds[bc],
                                et[:, bc, jj, lc * LCH : (lc + 1) * LCH],
                                start=(jp == 0),
                                stop=(jp == JS - 1),
                            )

            for i in range(NGRP + LEAD):
                if i < NGRP:
                    emit_a(i)
                if i == 0:
                    for ii in range(NGRP - N_PRE, NGRP):
                        pj0, pg = GROUP_STARTS[ii], GROUP_SIZES[ii]
                        pt = cpool.tile(
                            [P, NB, pg, MAX_LEN], BF16, tag=f"epre{ii}", name=f"epre{ii}"
                        )
                        nc.scalar.dma_start(out=pt, in_=e2[:, :, pj0 : pj0 + pg, :])
                        pre_tiles[ii] = pt
                if i >= LEAD:
                    emit_b(i - LEAD)

            # ---- tail: psum -> sbuf (bf16) -> dram, one DMA per b-chunk ----
            for bc in range(NB):
                o = opool.tile([P, MAX_LEN], BF16, tag=f"o{bc}", name=f"o{bc}")
                nc.vector.tensor_copy(o[:, :LCH], ps_out[bc][0])
                nc.scalar.copy(o[:, LCH:], ps_out[bc][1])
                nc.sync.dma_start(out=out[bc * P : (bc + 1) * P, :], in_=o)

    nc.compile()
    return nc


def get_nc():
    global _CACHED_NC
    if _CACHED_NC is None:
        _CACHED_NC = build_nc()
    return _CACHED_NC


def make_in_maps(emb_q, emb_iseq, w_f, b_f, w_h):
    """Host-side shard + layout + bf16 cast.  Returns list of per-core dicts."""
    q_t = np.ascontiguousarray(emb_q.astype(np.float32).T).astype(bf16_np)  # [k, b]
    qpart = q_t.reshape(KC, P, BSZ).transpose(1, 0, 2).reshape(P, KC * BSZ)
    wh_col = w_h.astype(bf16_np).reshape(1, D_ATTN).T  # [128, 1]
    in_maps = []
    for c in range(N_CORES):
        js, je = c * JS, (c + 1) * JS
        w_slice = w_f[js:je].reshape(JA, D)                       # [ja, k]
        w_t = w_slice.T.astype(bf16_np)                           # [k, ja]
        w2 = np.ascontiguousarray(
            w_t.reshape(KC, P, JA).transpose(1, 0, 2)
        )                                                         # [128, KC, ja]
        bias32 = np.ascontiguousarray(
            b_f[js:je].T.astype(np.float32)
        )                                                         # [a, j'] fp32
        bias_as_bf16 = bias32.view(bf16_np)                       # [128, 64]

        head = np.zeros((P, HEADC), dtype=bf16_np)
        head[:, :Q_COLS] = qpart
        head[:, Q_COLS : Q_COLS + 1] = wh_col
        head[:, BIAS_OFF : BIAS_OFF + 2 * JS] = bias_as_bf16
        head[:, W_OFF:ID_OFF] = w2[:, :, : HEAD_WJ * D_ATTN].reshape(
            P, KC * HEAD_WJ * D_ATTN
        )
        head[:, ID_OFF:] = np.eye(P, dtype=bf16_np)

        w_mid = np.ascontiguousarray(
            w2[:, :, HEAD_WJ * D_ATTN : (HEAD_WJ + MID_WJ) * D_ATTN]
        )
        w_tail = np.ascontiguousarray(w2[:, :, (HEAD_WJ + MID_WJ) * D_ATTN :])

        e_perm = emb_iseq[:, :, js:je].transpose(0, 2, 1)         # [b, j', l]
        e2 = np.ascontiguousarray(
            e_perm.astype(bf16_np)
            .reshape(NB, P, JS, MAX_LEN)
            .transpose(1, 0, 2, 3)
        )                                                         # [128, NB, j', l]
        in_maps.append({"head": head, "w_mid": w_mid, "w_tail": w_tail, "e2": e2})
    return in_maps


def run(in_maps, trace=False, **kwargs):
    nc = get_nc()
    return run_bass_kernel_spmd(
        nc, in_maps, core_ids=list(range(N_CORES)), trace=trace, **kwargs
    )


def kernel(emb_q, emb_iseq, w_f, b_f, w_h):
    emb_q, emb_iseq, w_f, b_f, w_h = (
        np.asarray(x) for x in (emb_q, emb_iseq, w_f, b_f, w_h)
    )
    in_maps = make_in_maps(emb_q, emb_iseq, w_f, b_f, w_h)
    res = run(in_maps, trace=False)
    partial = np.zeros((BSZ, MAX_LEN), dtype=np.float32)
    for r in res.results:
        partial += r["out"].astype(np.float32)
    return partial
